# revision 33
# baseline (speedup 1.0000x reference)
"""Trainium2 Bass kernel for nn_FLASH_ShareA_FFConvM.

Strategy: data-parallel over (batch, seq-half): 8 cores, each handling 4096
tokens (16 local-attention chunks of 256). All weights replicated. Per core:

  phase A (token-major, in blocks of ~11 tiles): LayerNorm stats with ONE
    batched sqrt per block (avoids ACT table thrash), normalized bf16 x
    stored token-major (xs_tok, reused for the residual add) and
    DMA-transposed into channel-major xsT [512c x 4224t] (col j = token
    j-128; 128-col halo at the front provides the token-shift source).
  phase B (per chunk pair): qk^T = wqk^T @ xs_sh^T; per-chunk mean ->
    OffsetScale with the softmax scale FOLDED into the q-side scales/offsets
    (host-side); q/k/qs/ks via DVE tensor_scalar (shift = col AP);
    sim^T[j,i] per head via PE with the causal mask ADDED VIA PE (identity
    matmul of a mask constant into the same PSUM accumulation);
    exp straight off PSUM in two wide [128,1024] ACT ops; softmax
    denominator via ones[128,128]-matmul (broadcasts the row-sum to all
    partitions, killing the DRAM-bounce broadcast); attn = exp * recip;
    V-matmul out^T[e,(h,i)]; silu -> og = silu * gate; fin[t,d] PSUM
    accumulates xs_sh@W_comb (folded hidden@w_out[:512]) + og@w_out[512:];
    finalize y = fin*silu(vgate) + xs_sh with the bf16 xs_tok copy
    (token-shift via small SBUF DMA).
  ACT ordering: exps for both chunks of a pair, then all silus -> 2 table
    loads per pair instead of ~5.
"""

import sys

sys.path.insert(0, "/opt/trn_rl_repo")

import numpy as np
import ml_dtypes
from contextlib import ExitStack

import concourse.bass as bass
import concourse.tile as tile
from concourse import bacc, mybir

F32 = mybir.dt.float32
BF16 = mybir.dt.bfloat16
FP8 = mybir.dt.float8e4
DR = mybir.MatmulPerfMode.DoubleRow
AX = mybir.AxisListType
ALU = mybir.AluOpType
ACTF = mybir.ActivationFunctionType

# fp8 quantization scales (dequant folded into ACT scale params)
KX = 8.0     # xs
KW = 256.0   # weights
KA = 128.0   # attn (folded into the ones-matmul as 1/KA)
KH = 16.0    # hidden
KO = KX      # og (must equal KX: shares the fin PSUM accumulation)
DQ = 1.0 / (KX * KW)          # == 1/(KA*KH); dequant for fin/vps/qk/vgate

B, SEQ, DIM = 4, 8192, 512
G, QK = 32, 128
CHUNK = SEQ // G          # 256 tokens per attention chunk
HD = QK // 4              # 32 (softmax scale dim, per source)
SCALE = float(HD) ** -0.5
HID = DIM                 # 512
EPS = 1e-5
N_CORES = 8
T_CORE = SEQ // 2         # 4096 tokens per core
NEG = -1.0e30

BF = ml_dtypes.bfloat16

SILU_NATIVE = True  # False: Sigmoid+mul fallback (CoreSim lacks Silu)


def build_core_program(ctx: ExitStack, tc, aps, n_tok, apply_g, apply_b):
    """Emit the per-core program. aps: dict name -> bass.AP (DRAM)."""
    nc = tc.nc
    n_tiles = n_tok // 128            # 128-token tiles (excl. halo tile)
    n_chunks = n_tok // CHUNK
    nt_all = n_tiles + 1              # + halo tile (rows 0..127 = token j-128)

    xp = aps["xp"]; yout = aps["y"]

    # ---------------- pools ----------------
    consts = ctx.enter_context(tc.tile_pool(name="consts", bufs=1))
    persist = ctx.enter_context(tc.tile_pool(name="persist", bufs=1))
    work = ctx.enter_context(tc.tile_pool(name="work", bufs=1))
    psum = ctx.enter_context(tc.tile_pool(name="psum", bufs=1, space="PSUM"))

    # ---------------- constants into SBUF ----------------
    def cload(name, shape, dtype):
        """Load a [R, C] DRAM const into SBUF; R>128 folds to [128, R//128, C]
        (row r = cc*128 + p -> tile[p, cc, :]), returning slices via [:, cc, :].
        """
        if shape[0] > 128:
            k = shape[0] // 128
            t = consts.tile([128, k, shape[1]], dtype, name=f"c_{name}",
                            tag=f"c_{name}")
            nc.sync.dma_start(t[:], aps[name].rearrange("(k p) c -> p k c",
                                                        p=128))
        else:
            t = consts.tile(shape, dtype, name=f"c_{name}", tag=f"c_{name}")
            nc.sync.dma_start(t[:], aps[name])
        return t

    wqk = cload("wqk", [512, 128], FP8)       # [128c, 4cc, 128d], x KW
    whvg = cload("whvg", [512, 1024], FP8)    # moving: cols 0:512 hid, 512: vgate
    wga = cload("wga", [512, 512], FP8)       # attn gate; lhsT tiles
    wcomb = cload("wcomb", [512, 512], FP8)   # moving rhs for fin part1
    woa = cload("woa", [2048, 512], BF16)     # moving rhs for fin part2, x KW
    g8 = cload("g8", [128, 7], F32)           # gammas.T (scaled, see host)
    b8 = cload("b8", [128, 7], F32)           # betas.T (scaled)
    bmask = cload("bmask", [128, 2048], BF16)  # [p,(jt,4h,256i)] 0/1 causal
    if apply_g:
        lng = cload("lng", [128, 512], F32)
    if apply_b:
        lnb = cload("lnb", [128, 512], F32)

    ones128 = consts.tile([128, 128], BF16, name="ones128", tag="ones128")
    nc.vector.memset(ones128[:], 1.0 / KA)
    epsb = consts.tile([128, 1], F32, name="epsb", tag="epsb")
    nc.vector.memset(epsb[:], EPS)

    # ---------------- persistent state ----------------
    # channel-major normalized x, fp8 (x KX); col j = token (j - 128)
    # folded: xsT[p, cc, j] = KX * xs[token j-128, channel cc*128+p]
    xsT = persist.tile([128, 4, n_tok + 128], FP8, name="xsT", tag="xsT")
    # token-major normalized x, bf16; slot i = tokens [(i-1)*128, i*128)
    xs_tok = persist.tile([128, nt_all, 512], BF16, name="xs_tok",
                          tag="xs_tok")
    # per-LN-tile stats: [:, i, 0] = mean, [:, i, 1] = rstd
    stats = persist.tile([128, nt_all, 2], F32, name="stats", tag="stats")

    def act(bi):
        """Chain ACT-table ops into a fixed order to avoid table thrash."""
        tc.chain_iter_dep("actfn", bi.ins)
        return bi

    # ---------------- phase A: LN in blocks (batched sqrt) ----------------
    x_keep = {}

    def stats_load(i):
        x_t = work.tile([128, 512], F32, name=f"xa{i}", tag="xa", bufs=16)
        nc.sync.dma_start(x_t[:], xp[i * 128:(i + 1) * 128, :])
        x_keep[i] = x_t
        bns = work.tile([128, 6], F32, name=f"bns{i}", tag="bns", bufs=2)
        nc.vector.bn_stats(out=bns[:], in_=x_t[:])
        nc.vector.bn_aggr(out=stats[:, i, :], in_=bns[:])

    def stats_finish(i0, i1):
        # batched rstd = 1/sqrt(var+eps) over the block's var column
        vs = stats[:, i0:i1, 1:2]
        act(nc.scalar.activation(out=vs, in_=vs, func=ACTF.Sqrt,
                                 bias=epsb[:]))
        nc.vector.reciprocal(out=vs, in_=vs)

    drip = []

    def drip_stats(n):
        for _ in range(min(n, len(drip))):
            stats_load(drip.pop(0))

    def emit_ln_tile(i):
        x_t = x_keep.pop(i)
        mean = stats[:, i, 0:1]
        rstd = stats[:, i, 1:2]
        dst = xs_tok[:, i, :]
        if apply_g or apply_b:
            xs_f = work.tile([128, 512], F32, name=f"xsf{i}", tag="xsf",
                             bufs=2)
            nc.vector.tensor_scalar(out=xs_f[:], in0=x_t[:],
                                    scalar1=mean, scalar2=rstd,
                                    op0=ALU.subtract, op1=ALU.mult)
            if apply_g and apply_b:
                nc.vector.tensor_mul(out=xs_f[:], in0=xs_f[:], in1=lng[:])
                nc.vector.tensor_add(out=dst, in0=xs_f[:], in1=lnb[:])
            elif apply_g:
                nc.vector.tensor_mul(out=dst, in0=xs_f[:], in1=lng[:])
            else:
                nc.vector.tensor_add(out=dst, in0=xs_f[:], in1=lnb[:])
        else:
            nc.vector.tensor_scalar(out=dst, in0=x_t[:],
                                    scalar1=mean, scalar2=rstd,
                                    op0=ALU.subtract, op1=ALU.mult)
        # transpose [128t, 512c] bf16 -> scratch, then cast (x KX) into fp8
        # xsT (DMA transpose cannot move 1-byte dtypes)
        scr = work.tile([128, 512], BF16, name=f"scr{i}", tag="scr", bufs=2)
        nc.sync.dma_start(scr.rearrange("p (c t) -> p c t", c=4), dst,
                          transpose=True)
        nc.vector.tensor_scalar_mul(
            out=xsT[:, :, i * 128:(i + 1) * 128],
            in0=scr.rearrange("p (c t) -> p c t", c=4), scalar1=KX)

    # stats: block 0 upfront; later blocks' loads DRIP through earlier
    # pairs' sections, with the batched sqrt at the consuming pair's start
    n_pairs = n_chunks // 2
    b0_end = min(11, nt_all)
    b1_end = min(23, nt_all)
    drip_sched = {}
    finish_at = {}
    if b1_end > b0_end:
        drip_sched[0] = list(range(b0_end, b1_end))
        finish_at[min(1, n_pairs - 1)] = (b0_end, b1_end)
    if nt_all > b1_end:
        drip_sched[min(2, n_pairs - 1)] = list(range(b1_end, nt_all))
        finish_at[min(3, n_pairs - 1)] = (b1_end, nt_all)

    ln_next = 0

    def ensure_ln(upto):
        nonlocal ln_next
        while ln_next <= min(upto, n_tiles):
            emit_ln_tile(ln_next)
            ln_next += 1

    def xsh8(cp, col0, width):
        """fp8 xs [128, 2, width] for cc-pair cp (0: shifted half, 1: rest)."""
        c0 = col0 - 1 if cp == 0 else col0
        return xsT[:, 2 * cp:2 * cp + 2, c0:c0 + width]

    def emit_pair_head(gp):
        """qk^T for the pair [128qk, 512] + gate^T (channel-major)."""
        colP = 128 + gp * 2 * CHUNK      # pair start col (unshifted)
        qkps = psum.tile([128, 512], F32, name=f"qkps{gp}", tag="mm",
                         bufs=6)
        for cp in range(2):
            nc.tensor.matmul(qkps[:], wqk[:, 2 * cp:2 * cp + 2, :],
                             xsh8(cp, colP, 512),
                             start=(cp == 0), stop=(cp == 1), perf_mode=DR)
        qkT = work.tile([128, 512], BF16, name=f"qkT{gp}", tag="qkT", bufs=2)
        act(nc.scalar.mul(out=qkT[:], in_=qkps[:], mul=DQ))

        gate_bf = []
        for ee in range(4):
            gps = psum.tile([128, 512], F32, name=f"g{gp}_{ee}", tag="mm",
                            bufs=6)
            for cp in range(2):
                nc.tensor.matmul(gps[:],
                                 wga[:, 2 * cp:2 * cp + 2,
                                     ee * 128:(ee + 1) * 128],
                                 xsh8(cp, colP, 512),
                                 start=(cp == 0), stop=(cp == 1), perf_mode=DR)
            gb = work.tile([128, 512], BF16, name=f"gate{gp}_{ee}",
                           tag=f"gate{ee}", bufs=2)
            # gate stored x KO so og lands at KO*og for the fin matmul
            act(nc.scalar.mul(out=gb[:], in_=gps[:], mul=KO * DQ))
            gate_bf.append(gb)
        return qkT, gate_bf

    # ---------------- phase B: chunk pairs (heads pipelined 1 ahead) ----
    for i in range(b0_end):
        stats_load(i)
    stats_finish(0, b0_end)
    heads = {}
    for gp in range(n_pairs):
        if gp in finish_at:
            stats_finish(*finish_at[gp])
        ensure_ln(4 * gp + 4)
        if gp in drip_sched:
            drip = drip_sched[gp]
        if gp not in heads:
            heads[gp] = emit_pair_head(gp)
        qkT, gate_bf = heads.pop(gp)

        # --- B/C) fronts: offsets, q/k/qs/ks, sim + mask + exp ---
        expt_g = []
        for g in (2 * gp, 2 * gp + 1):
            half = g % 2
            qk_c = qkT[:, half * 256:(half + 1) * 256]

            qsum = work.tile([128, 1], F32, name=f"qsum{g}", tag="qsum",
                             bufs=2)
            nc.vector.tensor_reduce(out=qsum[:], in_=qk_c, axis=AX.X,
                                    op=ALU.add)
            offs = work.tile([128, 7], F32, name=f"offs{g}", tag="offs",
                             bufs=2)
            # offs[:, i] = qk_sum * gamma_i/CHUNK + beta_i  (one DVE op)
            # cols: 0 qsc*S, 1 qoff*S, 2 qsc, 3 qsoff*S, 4 ksc, 5 koff, 6 ksoff
            nc.vector.scalar_tensor_tensor(out=offs[:], in0=g8[:],
                                           scalar=qsum[:], in1=b8[:],
                                           op0=ALU.mult, op1=ALU.add)

            qT = work.tile([128, 256], BF16, name=f"qT{g}", tag="qT", bufs=2)
            kT = work.tile([128, 256], BF16, name=f"kT{g}", tag="kT", bufs=2)
            nc.vector.tensor_scalar(out=qT[:], in0=qk_c,
                                    scalar1=offs[:, 0:1], scalar2=offs[:, 1:2],
                                    op0=ALU.mult, op1=ALU.add)
            nc.vector.tensor_scalar(out=kT[:], in0=qk_c,
                                    scalar1=offs[:, 4:5], scalar2=offs[:, 5:6],
                                    op0=ALU.mult, op1=ALU.add)
            qsT = work.tile([128, 256], BF16, name=f"qsT{g}", tag="qsT",
                            bufs=2)
            ksT = work.tile([128, 256], BF16, name=f"ksT{g}", tag="ksT",
                            bufs=2)
            nc.vector.tensor_copy(out=qsT[:, 0:1], in_=offs[:, 3:4])
            nc.vector.tensor_copy(out=ksT[:, 0:1], in_=offs[:, 6:7])
            nc.vector.tensor_scalar(out=qsT[:, 1:256], in0=qT[:, 0:255],
                                    scalar1=offs[:, 2:3], scalar2=offs[:, 3:4],
                                    op0=ALU.mult, op1=ALU.add)
            nc.vector.tensor_scalar(out=ksT[:, 1:256], in0=kT[:, 0:255],
                                    scalar1=offs[:, 4:5], scalar2=offs[:, 6:7],
                                    op0=ALU.mult, op1=ALU.add)

            # sim^T per j-tile: [128j, 4h*256i]; mask added via PE
            expt = [work.tile([128, 1024], BF16, name=f"exp{g}_{jt}",
                              tag=f"exp{jt}", bufs=2) for jt in range(2)]
            for jt in range(2):
                for h in range(4):
                    Q = qT if h < 2 else qsT
                    K = kT if h < 2 else ksT
                    dr = (h % 2) * 64
                    simx = psum.tile([128, 256], F32,
                                     name=f"sim{g}_{jt}_{h}", tag="mm",
                                     bufs=6)
                    nc.tensor.matmul(
                        simx[:], K[dr:dr + 64, jt * 128:(jt + 1) * 128],
                        Q[dr:dr + 64, :], start=True, stop=True)
                    act(nc.scalar.activation(
                        out=expt[jt][:, h * 256:(h + 1) * 256],
                        in_=simx[:], func=ACTF.Exp))
                # causal mask as 0/1 multiply (logits are small: no overflow)
                nc.vector.tensor_mul(
                    out=expt[jt][:], in0=expt[jt][:],
                    in1=bmask[:, jt * 1024:(jt + 1) * 1024])
            expt_g.append(expt)

        if gp + 1 < n_pairs:
            ensure_ln(4 * gp + 8)
            heads[gp + 1] = emit_pair_head(gp + 1)

        # --- D/E) mids: denominators + attn; hidden/vgate matmuls ---
        attn_g = []
        hv_ps = []
        for gi, g in enumerate((2 * gp, 2 * gp + 1)):
            expt = expt_g[gi]
            recb = work.tile([128, 1024], F32, name=f"recb{g}", tag="recb",
                             bufs=2)
            for s in range(2):
                sums = psum.tile([128, 512], F32, name=f"sums{g}_{s}",
                                 tag="acc", bufs=2)
                for jt in range(2):
                    nc.tensor.matmul(sums[:], ones128[:],
                                     expt[jt][:, s * 512:(s + 1) * 512],
                                     start=(jt == 0), stop=(jt == 1))
                nc.vector.reciprocal_approx_fast(
                    out=recb[:, s * 512:(s + 1) * 512], in_=sums[:])
            # attn8 [128, 2jt, 1024] fp8, values KA*attn (KA folded via ones)
            attn = work.tile([128, 2, 1024], FP8, name=f"attn{g}",
                             tag="attn", bufs=2)
            for jt in range(2):
                nc.vector.tensor_mul(out=attn[:, jt, :], in0=expt[jt][:],
                                     in1=recb[:])
            attn_g.append(attn)

            # hidden + vgate matmuls for the chunk's 2 t-tiles (PE filler);
            # cols 0:512 hidden, 512:1024 vgate in one 2-bank tile
            colC = 128 + g * CHUNK
            ps = []
            for tt in range(2):
                colT = colC + tt * 128
                halves = [psum.tile([128, 512], F32,
                                    name=f"hv{g}_{tt}_{s}", tag="mm",
                                    bufs=6)[:] for s in range(2)]
                for s in range(2):
                    for cp in range(2):
                        nc.tensor.matmul(halves[s],
                                         xsh8(cp, colT, 128),
                                         whvg[:, 2 * cp:2 * cp + 2,
                                              s * 512:(s + 1) * 512],
                                         start=(cp == 0), stop=(cp == 1),
                                         perf_mode=DR)
                ps.append(halves)
            hv_ps.append(ps)

        # hid copies + vgate silus for BOTH chunks (frees the hv PSUM slots
        # before the V matmuls rotate into them). hid8: [128, 2jt, 512e] fp8
        # (x KH) so the V matmul can contract both j-tiles in one DoubleRow.
        hid_g = []
        svg_g = []
        for gi, g in enumerate((2 * gp, 2 * gp + 1)):
            hid8 = work.tile([128, 2, 512], FP8, name=f"hid{g}", tag="hid",
                             bufs=2)
            svg_bf = []
            for tt in range(2):
                hvh, hvv2 = hv_ps[gi][tt]
                act(nc.scalar.mul(out=hid8[:, tt, :], in_=hvh,
                                  mul=KH * DQ))
                sv = work.tile([128, 512], BF16, name=f"svg{g}_{tt}",
                               tag="svg", bufs=4)
                if SILU_NATIVE:
                    act(nc.scalar.activation(out=sv[:], in_=hvv2,
                                             func=ACTF.Silu, scale=DQ))
                else:
                    sgt = work.tile([128, 512], F32, name=f"sgt{g}_{tt}",
                                    tag="sgt", bufs=2)
                    act(nc.scalar.activation(out=sgt[:], in_=hvv2,
                                             func=ACTF.Sigmoid, scale=DQ))
                    nc.vector.scalar_tensor_tensor(
                        out=sv[:], in0=hvv2, scalar=DQ,
                        in1=sgt[:], op0=ALU.mult, op1=ALU.mult)
                svg_bf.append(sv)
            hid_g.append(hid8)
            svg_g.append(svg_bf)

        # --- F/G) backs: V, og, fin, finalize ---
        for gi, g in enumerate((2 * gp, 2 * gp + 1)):
            half = g % 2
            attn = attn_g[gi]
            hid8 = hid_g[gi]
            svg_bf = svg_g[gi]

            og_bf = [work.tile([128, 1024], BF16, name=f"og{g}_{ee}",
                               tag=f"og{ee}", bufs=2) for ee in range(4)]
            for ee in range(4):
                vsubs = [psum.tile([128, 512], F32,
                                   name=f"v{g}_{ee}_{s}", tag="mm",
                                   bufs=6)[:] for s in range(2)]
                for s in range(2):
                    nc.tensor.matmul(
                        vsubs[s],
                        hid8[:, :, ee * 128:(ee + 1) * 128],
                        attn[:, :, s * 512:(s + 1) * 512],
                        start=True, stop=True, perf_mode=DR)
                osl = work.tile([128, 1024], BF16, name=f"osl{g}_{ee}",
                                tag="osl", bufs=2)
                for s in range(2):
                    sl = slice(s * 512, (s + 1) * 512)
                    if SILU_NATIVE:
                        act(nc.scalar.activation(out=osl[:, sl],
                                                 in_=vsubs[s],
                                                 func=ACTF.Silu, scale=DQ))
                    else:
                        sgo = work.tile([128, 512], F32,
                                        name=f"sgo{g}_{ee}_{s}",
                                        tag="sgo", bufs=2)
                        act(nc.scalar.activation(out=sgo[:], in_=vsubs[s],
                                                 func=ACTF.Sigmoid, scale=DQ))
                        nc.vector.scalar_tensor_tensor(
                            out=osl[:, sl], in0=vsubs[s], scalar=DQ,
                            in1=sgo[:], op0=ALU.mult, op1=ALU.mult)
                gslice = gate_bf[ee][:, half * 256:(half + 1) * 256]
                gbc = gslice.unsqueeze(1).broadcast_to((128, 4, 256))
                nc.vector.tensor_mul(
                    out=og_bf[ee].rearrange("p (h i) -> p h i", h=4),
                    in0=osl.rearrange("p (h i) -> p h i", h=4),
                    in1=gbc)

            for tt in range(2):
                ti = g * 2 + tt
                colT = 128 + ti * 128
                fin = psum.tile([128, 512], F32, name=f"fin{g}_{tt}",
                                tag="acc", bufs=2)
                for cp in range(2):
                    nc.tensor.matmul(fin[:], xsh8(cp, colT, 128),
                                     wcomb[:, 2 * cp:2 * cp + 2, :],
                                     start=(cp == 0), stop=False,
                                     perf_mode=DR)
                for h in range(4):
                    for ee in range(4):
                        ff = h * 4 + ee
                        nc.tensor.matmul(
                            fin[:],
                            og_bf[ee][:, h * 256 + tt * 128:
                                      h * 256 + tt * 128 + 128],
                            woa[:, ff, :],
                            start=False, stop=(ff == 15))

                xsprev = work.tile([128, 256], BF16, name=f"xsp{ti}",
                                   tag="xsp", bufs=2)
                nc.sync.dma_start(xsprev[1:128, :],
                                  xs_tok[0:127, ti + 1, 0:256])
                nc.sync.dma_start(xsprev[0:1, :],
                                  xs_tok[127:128, ti, 0:256])

                y = work.tile([128, 512], F32, name=f"y{ti}", tag="y",
                              bufs=3)
                nc.vector.scalar_tensor_tensor(
                    out=y[:], in0=fin[:], scalar=DQ, in1=svg_bf[tt][:],
                    op0=ALU.mult, op1=ALU.mult)
                nc.vector.tensor_add(out=y[:, 256:512], in0=y[:, 256:512],
                                     in1=xs_tok[:, ti + 1, 256:512])
                nc.vector.tensor_add(out=y[:, 0:256], in0=y[:, 0:256],
                                     in1=xsprev[:])
                nc.sync.dma_start(yout[ti * 128:(ti + 1) * 128, :], y[:])
            drip_stats(6)


def make_host_inputs(x, ln_g, ln_b, w_qk, g4, b4, g2, b2, w_hidden, w_gate,
                     w_out, n_tok=T_CORE):
    """Build shared weight arrays + per-core xp slices."""
    x = np.asarray(x, np.float32)
    ln_g = np.asarray(ln_g, np.float32)
    ln_b = np.asarray(ln_b, np.float32)
    apply_g = not np.all(ln_g == 1.0)
    apply_b = bool(np.any(ln_b != 0.0))

    w_hidden = np.asarray(w_hidden, np.float32)
    w_out = np.asarray(w_out, np.float32)
    w_gate = np.asarray(w_gate, np.float32)
    w_qk = np.asarray(w_qk, np.float32)

    wcomb = (w_hidden[:, :HID] @ w_out[:HID, :]).astype(np.float32)

    # offset/scale gammas+betas with softmax scale folded into the q side.
    # cols: 0 qsc*S, 1 qoff*S, 2 qsc, 3 qsoff*S, 4 ksc, 5 koff, 6 ksoff
    g4n = np.asarray(g4, np.float32) / CHUNK     # [4, 128] rows q_off,k_off,q_sc,k_sc
    b4n = np.asarray(b4, np.float32)
    g2n = np.asarray(g2, np.float32) / CHUNK     # rows q_s_off, k_s_off
    b2n = np.asarray(b2, np.float32)
    S = SCALE
    g8 = np.stack([g4n[2] * S, g4n[0] * S, g4n[2], g2n[0] * S,
                   g4n[3], g4n[1], g2n[1]], axis=1).copy()   # [128, 7]
    b8 = np.stack([b4n[2] * S, b4n[0] * S, b4n[2], b2n[0] * S,
                   b4n[3], b4n[1], b2n[1]], axis=1).copy()

    # 0/1 causal mask, keys-major: bmask[p, (jt, h, i)] = jt*128+p <= i
    jj, ii = np.meshgrid(np.arange(256), np.arange(256), indexing="ij")
    maskt = np.where(jj > ii, np.float32(0.0), np.float32(1.0))  # [256j,256i]
    m4 = maskt.reshape(2, 128, 256).transpose(1, 0, 2)           # [128p,2jt,256i]
    bmask = np.concatenate([m4] * 4, axis=2).reshape(128, 2048)  # dup over 4h

    E4 = ml_dtypes.float8_e4m3fn

    def q8(w):
        return np.clip(np.asarray(w, np.float32) * KW, -240, 240).astype(E4)

    shared = {
        "wqk": q8(w_qk),
        "whvg": q8(np.concatenate([w_hidden[:, :HID], w_gate], axis=1)),
        "wga": q8(w_hidden[:, HID:]),
        "wcomb": q8(wcomb),
        "woa": (np.asarray(w_out[HID:, :], np.float32) * KW).astype(BF),
        "g8": g8,
        "b8": b8,
        "bmask": bmask.astype(BF),
    }
    if apply_g:
        shared["lng"] = np.broadcast_to(ln_g, (128, DIM)).copy()
    if apply_b:
        shared["lnb"] = np.broadcast_to(ln_b, (128, DIM)).copy()

    n_half = x.shape[1] // n_tok  # halves per batch row
    per_core = []
    for core in range(x.shape[0] * n_half):
        b = core // n_half
        h = core % n_half
        t0 = h * n_tok
        xp = np.zeros((n_tok + 128, DIM), np.float32)
        xp[128:] = x[b, t0:t0 + n_tok]
        if t0 > 0:
            xp[127] = x[b, t0 - 1]
        per_core.append({"xp": xp})
    return shared, per_core, apply_g, apply_b


def build_bass(n_tok, apply_g, apply_b):
    nc = bacc.Bacc("TRN2", target_bir_lowering=False, debug=False,
                   num_devices=1)
    specs = {
        "xp": ([n_tok + 128, DIM], F32),
        "wqk": ([512, 128], FP8),
        "whvg": ([512, 1024], FP8),
        "wga": ([512, 512], FP8),
        "wcomb": ([512, 512], FP8),
        "woa": ([2048, 512], BF16),
        "g8": ([128, 7], F32),
        "b8": ([128, 7], F32),
        "bmask": ([128, 2048], BF16),
    }
    if apply_g:
        specs["lng"] = ([128, 512], F32)
    if apply_b:
        specs["lnb"] = ([128, 512], F32)
    aps = {}
    for name, (shape, dt) in specs.items():
        aps[name] = nc.dram_tensor(name, shape, dt, kind="ExternalInput").ap()
    aps["y"] = nc.dram_tensor("y", [n_tok, DIM], F32,
                              kind="ExternalOutput").ap()

    with tile.TileContext(nc) as tc:
        with ExitStack() as ctx:
            build_core_program(ctx, tc, aps, n_tok, apply_g, apply_b)
    nc.compile()
    return nc


def _run(inputs, trace=False, **spmd_kwargs):
    from concourse.bass_utils import run_bass_kernel_spmd

    shared, per_core, apply_g, apply_b = make_host_inputs(
        inputs["x"], inputs["ln_g"], inputs["ln_b"], inputs["w_qk"],
        inputs["g4"], inputs["b4"], inputs["g2"], inputs["b2"],
        inputs["w_hidden"], inputs["w_gate"], inputs["w_out"])

    nc = build_bass(T_CORE, apply_g, apply_b)

    in_maps = [{**shared, **pc} for pc in per_core]
    res = run_bass_kernel_spmd(nc, in_maps, core_ids=list(range(N_CORES)),
                               trace=trace, **spmd_kwargs)

    y = np.empty((B, SEQ, DIM), np.float32)
    n_half = SEQ // T_CORE
    for core in range(N_CORES):
        b = core // n_half
        h = core % n_half
        y[b, h * T_CORE:(h + 1) * T_CORE] = res.results[core]["y"]
    return y, res


def kernel(**inputs):
    return _run(inputs)[0]
